# revision 5
# baseline (speedup 1.0000x reference)
"""MiniBatchSemiNMF encode kernel for Trainium2 (8 NeuronCores, Bass/Tile).

Data-parallel over the batch (rows of `acts`): each of the 8 cores gets
1024 rows; D-derived k x k cache terms (ddt_pos, ddt_neg, (ddt+eps I)^-1)
are computed on the host (tiny: 512x512) and replicated to every core.

Device computation per core, in a transposed layout (k on partitions,
rows on the free dim), so no on-device transposes are needed:
    atdT  = D @ actsT                                  (PE, fp32r)
    z0T   = max(inv @ atdT, eps)                       (PE + DVE)
    loop: numT = atd_posT     + ddt_neg @ zT           (PE, identity-matmul
          denT = atd_negT+eps + ddt_pos @ zT            folds the adds into
          zT  *= sqrt(numT) * rsqrt(denT)               the PSUM group)
Iteration matmuls run as fp32r (full PE rate; ~12-bit mantissa products,
fp32 accumulate) -- iteration noise is contracted by the dynamics. The
atd/z0 matmuls run in exact fp32 (4x cycles, but only 96 of 1696 MMs):
their rounding would persist in z as an initial-condition error. Elementwise runs on ACT (sqrt/rsqrt/relu) and DVE (mul/add).
"""

import sys

for _p in ("/opt/trn_rl_repo",):
    if _p not in sys.path:
        sys.path.insert(0, _p)

import numpy as np

import concourse.bacc as bacc
import concourse.tile as tile
from concourse import mybir
from concourse.bass_utils import run_bass_kernel_spmd

F32 = mybir.dt.float32
F32R = mybir.dt.float32r

EPS = 1e-8
N_CORES = 8
B, DM, K = 8192, 1024, 512  # batch, d_model, n_concepts
R = B // N_CORES  # rows per core (1024)
RC = 512  # row-chunk (moving-operand width)
NRC = R // RC  # 2 row chunks
NK = K // 128  # 4 k-tiles
ND = DM // 128  # 8 d-tiles

_BUILD_CACHE: dict[int, object] = {}  # v2: den-add on DVE


def _build(n_iters: int):
    """Build (and bacc-compile) the per-core Bass program."""
    nc = bacc.Bacc("TRN2", target_bir_lowering=False, debug=False, num_devices=N_CORES)

    actsT_d = nc.dram_tensor("actsT", [DM, R], F32, kind="ExternalInput").ap()
    DT_d = nc.dram_tensor("DT", [DM, K], F32, kind="ExternalInput").ap()
    dpos_d = nc.dram_tensor("ddt_pos", [K, K], F32R, kind="ExternalInput").ap()
    dneg_d = nc.dram_tensor("ddt_neg", [K, K], F32R, kind="ExternalInput").ap()
    inv_d = nc.dram_tensor("ddt_inv", [K, K], F32, kind="ExternalInput").ap()
    eye_d = nc.dram_tensor("eye", [128, 128], F32R, kind="ExternalInput").ap()
    out_d = nc.dram_tensor("zT", [K, R], F32, kind="ExternalOutput").ap()

    Relu = mybir.ActivationFunctionType.Relu
    Sqrt = mybir.ActivationFunctionType.Sqrt
    Rsqrt = mybir.ActivationFunctionType.Rsqrt
    Copy = mybir.ActivationFunctionType.Copy

    with tile.TileContext(nc) as tc:
        with (
            tc.tile_pool(name="weights", bufs=1) as wp,
            tc.tile_pool(name="big", bufs=1) as bigp,
            tc.tile_pool(name="zacts", bufs=2 * NK * NRC) as zap,
            tc.tile_pool(name="tmp", bufs=4) as tmpp,
            tc.tile_pool(name="psum", bufs=4, space="PSUM") as psp,
        ):
            # --- persistent weights ---
            eye_sb = wp.tile([128, 128], F32R, name="eye_sb", tag="eye")
            nc.sync.dma_start(eye_sb[:], eye_d[:])
            DT_sb = []
            for d in range(ND):
                t = wp.tile([128, K], F32, name=f"DT_sb{d}", tag=f"DT{d}")
                nc.sync.dma_start(t[:], DT_d[d * 128 : (d + 1) * 128, :])
                DT_sb.append(t)
            # actsT tiles share slots with z tiles (tag "za"): actsT is dead
            # after phase 1, exactly when z starts being written.
            acts_sb = [[None] * NRC for _ in range(ND)]
            for d in range(ND):
                for rc in range(NRC):
                    t = zap.tile([128, RC], F32, name=f"acts{d}_{rc}", tag="za")
                    nc.sync.dma_start(
                        t[:], actsT_d[d * 128 : (d + 1) * 128, rc * RC : (rc + 1) * RC]
                    )
                    acts_sb[d][rc] = t
            inv_sb, dpos_sb, dneg_sb = [], [], []
            for k in range(NK):
                rows = slice(k * 128, (k + 1) * 128)
                t = wp.tile([128, K], F32, name=f"inv_sb{k}", tag=f"inv{k}")
                nc.sync.dma_start(t[:], inv_d[rows, :])
                inv_sb.append(t)
                t = wp.tile([128, K], F32R, name=f"dpos_sb{k}", tag=f"dpos{k}")
                nc.sync.dma_start(t[:], dpos_d[rows, :])
                dpos_sb.append(t)
                t = wp.tile([128, K], F32R, name=f"dneg_sb{k}", tag=f"dneg{k}")
                nc.sync.dma_start(t[:], dneg_d[rows, :])
                dneg_sb.append(t)

            # --- phase 1: atdT = D @ actsT, then relu splits ---
            atd_sb = [[None] * NRC for _ in range(NK)]
            pos_sb = [[None] * NRC for _ in range(NK)]
            negeps_sb = [[None] * NRC for _ in range(NK)]
            for kp in range(NK):
                cols = slice(kp * 128, (kp + 1) * 128)
                for rc in range(NRC):
                    ps = psp.tile([128, RC], F32, name=f"ps_atd{kp}_{rc}", tag="pn")
                    for d in range(ND):
                        nc.tensor.matmul(
                            ps[:],
                            DT_sb[d][:, cols],
                            acts_sb[d][rc][:],
                            start=(d == 0),
                            stop=(d == ND - 1),
                        )
                    atd = bigp.tile([128, RC], F32, name=f"atd{kp}_{rc}", tag=f"atd{kp}_{rc}")
                    nc.scalar.activation(atd[:], ps[:], Copy)
                    pos = bigp.tile([128, RC], F32R, name=f"pos{kp}_{rc}", tag=f"pos{kp}_{rc}")
                    nc.scalar.activation(pos[:], ps[:], Relu)
                    neg = tmpp.tile([128, RC], F32, name=f"neg{kp}_{rc}", tag="negt")
                    nc.scalar.activation(neg[:], ps[:], Relu, scale=-1.0)
                    nege = bigp.tile(
                        [128, RC], F32R, name=f"nege{kp}_{rc}", tag=f"nege{kp}_{rc}"
                    )
                    nc.vector.tensor_scalar_add(nege[:], neg[:], EPS)
                    atd_sb[kp][rc] = atd
                    pos_sb[kp][rc] = pos
                    negeps_sb[kp][rc] = nege

            # --- phase 2: z0T = max(inv @ atdT, eps) ---
            z_sb = [[[None] * NRC for _ in range(NK)] for _ in range(2)]
            for p in range(2):
                for k in range(NK):
                    for rc in range(NRC):
                        z_sb[p][k][rc] = zap.tile(
                            [128, RC], F32R, name=f"z{p}_{k}_{rc}", tag="za"
                        )
            for kp in range(NK):
                cols = slice(kp * 128, (kp + 1) * 128)
                for rc in range(NRC):
                    ps = psp.tile([128, RC], F32, name=f"ps_z0{kp}_{rc}", tag="pd")
                    for k in range(NK):
                        nc.tensor.matmul(
                            ps[:],
                            inv_sb[k][:, cols],
                            atd_sb[k][rc][:],
                            start=(k == 0),
                            stop=(k == NK - 1),
                        )
                    nc.vector.tensor_scalar_max(z_sb[0][kp][rc][:], ps[:], EPS)

            # --- phase 3: multiplicative updates ---
            for t_it in range(n_iters):
                cur, nxt = t_it % 2, (t_it + 1) % 2
                for rc in range(NRC):
                    for kp in range(NK):
                        cols = slice(kp * 128, (kp + 1) * 128)
                        pn = psp.tile(
                            [128, RC], F32, name=f"pn{t_it}_{rc}_{kp}", tag="pn"
                        )
                        nc.tensor.matmul(
                            pn[:], eye_sb[:], pos_sb[kp][rc][:], start=True, stop=False
                        )
                        for k in range(NK):
                            nc.tensor.matmul(
                                pn[:],
                                dneg_sb[k][:, cols],
                                z_sb[cur][k][rc][:],
                                start=False,
                                stop=(k == NK - 1),
                            )
                        pd = psp.tile(
                            [128, RC], F32, name=f"pd{t_it}_{rc}_{kp}", tag="pd"
                        )
                        for k in range(NK):
                            nc.tensor.matmul(
                                pd[:],
                                dpos_sb[k][:, cols],
                                z_sb[cur][k][rc][:],
                                start=(k == 0),
                                stop=(k == NK - 1),
                            )
                        den = tmpp.tile(
                            [128, RC], F32, name=f"den{t_it}_{rc}_{kp}", tag="den"
                        )
                        nc.vector.tensor_add(den[:], pd[:], negeps_sb[kp][rc][:].bitcast(F32))
                        rcp = tmpp.tile(
                            [128, RC], F32, name=f"rcp{t_it}_{rc}_{kp}", tag="rcp"
                        )
                        nc.vector.reciprocal_approx_fast(rcp[:], den[:])
                        rat = tmpp.tile(
                            [128, RC], F32, name=f"rat{t_it}_{rc}_{kp}", tag="rat"
                        )
                        nc.vector.tensor_mul(rat[:], pn[:], rcp[:])
                        f = tmpp.tile([128, RC], F32, name=f"f{t_it}_{rc}_{kp}", tag="f")
                        nc.scalar.activation(f[:], rat[:], Sqrt)
                        nc.gpsimd.tensor_mul(
                            z_sb[nxt][kp][rc][:],
                            z_sb[cur][kp][rc][:].bitcast(F32),
                            f[:],
                        )

            # --- output ---
            fin = n_iters % 2
            for kp in range(NK):
                for rc in range(NRC):
                    nc.sync.dma_start(
                        out_d[kp * 128 : (kp + 1) * 128, rc * RC : (rc + 1) * RC],
                        z_sb[fin][kp][rc][:].bitcast(F32),
                    )

    nc.compile()
    return nc


def _get_program(n_iters: int):
    if n_iters not in _BUILD_CACHE:
        _BUILD_CACHE[n_iters] = _build(n_iters)
    return _BUILD_CACHE[n_iters]


def make_in_maps(acts: np.ndarray, D: np.ndarray):
    """Host-side sharding + kxk cache terms."""
    acts = np.ascontiguousarray(acts, dtype=np.float32)
    D = np.ascontiguousarray(D, dtype=np.float32)
    ddt = D @ D.T
    ddt_pos = ((np.abs(ddt) + ddt) * 0.5).astype(np.float32)
    ddt_neg = ((np.abs(ddt) - ddt) * 0.5).astype(np.float32)
    eye_k = np.eye(K, dtype=np.float32)
    inv = np.linalg.solve(ddt + np.float32(EPS) * eye_k, eye_k).astype(np.float32)
    DT = np.ascontiguousarray(D.T)
    actsT = np.ascontiguousarray(acts.T)
    eye128 = np.eye(128, dtype=np.float32)
    in_maps = []
    for c in range(N_CORES):
        in_maps.append(
            {
                "actsT": np.ascontiguousarray(actsT[:, c * R : (c + 1) * R]),
                "DT": DT,
                "ddt_pos": ddt_pos,
                "ddt_neg": ddt_neg,
                "ddt_inv": inv,
                "eye": eye128,
            }
        )
    return in_maps


def kernel(acts: np.ndarray, D: np.ndarray, n_iters) -> np.ndarray:
    n_iters = int(n_iters)
    nc = _get_program(n_iters)
    in_maps = make_in_maps(acts, D)
    res = run_bass_kernel_spmd(nc, in_maps, core_ids=list(range(N_CORES)))
    z = np.empty((B, K), dtype=np.float32)
    for c in range(N_CORES):
        z[c * R : (c + 1) * R, :] = res.results[c]["zT"].T
    return z


# revision 9
# speedup vs baseline: 1.7483x; 1.7483x over previous
"""MiniBatchSemiNMF encode kernel for Trainium2 (8 NeuronCores, Bass/Tile).

Data-parallel over the batch (rows of `acts`): each of the 8 cores gets
1024 rows; D-derived k x k cache terms (ddt_pos, ddt_neg, (ddt+eps I)^-1)
are computed on the host (tiny: 512x512) and replicated to every core.

Device computation per core, in a transposed layout (k on partitions,
rows on the free dim), so no on-device transposes are needed:
    atdT  = D @ actsT                                  (PE, exact fp32)
    z0T   = max(inv @ atdT, eps)                       (PE + DVE)
    loop: numT = atd_posT + ddt_neg @ zT     (PE; identity-matmul folds the
          denT = atd_negT+eps + ddt_pos @ zT    atd_pos add into the PSUM
          zT  *= sqrt(numT / denT)              group; den add on DVE)
Iteration matmuls run as fp32r (full PE rate; ~12-bit mantissa products,
fp32 accumulate) -- iteration noise is contracted by the dynamics. The
atd/z0 matmuls run in exact fp32 (4x cycles, but only 96 of 1696 MMs):
their rounding would persist in z as an initial-condition error
(heavy cancellation in acts@D.T and atd@inv amplifies it ~20x).
"""

import sys

for _p in ("/opt/trn_rl_repo",):
    if _p not in sys.path:
        sys.path.insert(0, _p)

import numpy as np

import concourse.bacc as bacc
import concourse.tile as tile
from concourse import mybir
from concourse.bass_utils import run_bass_kernel_spmd

F32 = mybir.dt.float32
F32R = mybir.dt.float32r

EPS = 1e-8
N_CORES = 8
B, DM, K = 8192, 1024, 512  # batch, d_model, n_concepts
R = B // N_CORES  # rows per core (1024)
RC = 512  # row-chunk (moving-operand width)
NRC = R // RC  # 2 row chunks
NK = K // 128  # 4 k-tiles
ND = DM // 128  # 8 d-tiles

_BUILD_CACHE: dict[int, object] = {}  # v2: den-add on DVE


def _build(n_iters: int):
    """Build (and bacc-compile) the per-core Bass program."""
    nc = bacc.Bacc("TRN2", target_bir_lowering=False, debug=False, num_devices=N_CORES)

    actsT_d = nc.dram_tensor("actsT", [DM, R], F32, kind="ExternalInput").ap()
    DT_d = nc.dram_tensor("DT", [DM, K], F32, kind="ExternalInput").ap()
    dpos_d = nc.dram_tensor("ddt_pos", [K, K], F32R, kind="ExternalInput").ap()
    dneg_d = nc.dram_tensor("ddt_neg", [K, K], F32R, kind="ExternalInput").ap()
    inv_d = nc.dram_tensor("ddt_inv", [K, K], F32, kind="ExternalInput").ap()
    eye_d = nc.dram_tensor("eye", [128, 128], F32R, kind="ExternalInput").ap()
    out_d = nc.dram_tensor("zT", [K, R], F32, kind="ExternalOutput").ap()

    Relu = mybir.ActivationFunctionType.Relu
    Sqrt = mybir.ActivationFunctionType.Sqrt
    Rsqrt = mybir.ActivationFunctionType.Rsqrt
    Copy = mybir.ActivationFunctionType.Copy

    with tile.TileContext(nc) as tc:
        with (
            tc.tile_pool(name="weights", bufs=1) as wp,
            tc.tile_pool(name="big", bufs=1) as bigp,
            tc.tile_pool(name="zacts", bufs=2 * NK * NRC) as zap,
            tc.tile_pool(name="tmp", bufs=4) as tmpp,
            tc.tile_pool(name="psum", bufs=4, space="PSUM") as psp,
        ):
            # --- persistent weights ---
            eye_sb = wp.tile([128, 128], F32R, name="eye_sb", tag="eye")
            nc.sync.dma_start(eye_sb[:], eye_d[:])
            DT_sb = []
            acts_sb = [[None] * NRC for _ in range(ND)]
            for d in range(ND):
                t = wp.tile([128, K], F32, name=f"DT_sb{d}", tag=f"DT{d}")
                nc.sync.dma_start(t[:], DT_d[d * 128 : (d + 1) * 128, :])
                DT_sb.append(t)
                t = zap.tile([128, RC], F32, name=f"acts{d}_0", tag="za")
                nc.sync.dma_start(t[:], actsT_d[d * 128 : (d + 1) * 128, 0:RC])
                acts_sb[d][0] = t
            for d in range(ND):
                for rc in range(1, NRC):
                    t = zap.tile([128, RC], F32, name=f"acts{d}_{rc}", tag="za")
                    nc.sync.dma_start(
                        t[:], actsT_d[d * 128 : (d + 1) * 128, rc * RC : (rc + 1) * RC]
                    )
                    acts_sb[d][rc] = t
            inv_sb, dpos_sb, dneg_sb = [], [], []
            for k in range(NK):
                rows = slice(k * 128, (k + 1) * 128)
                t = wp.tile([128, K], F32, name=f"inv_sb{k}", tag=f"inv{k}")
                nc.sync.dma_start(t[:], inv_d[rows, :])
                inv_sb.append(t)
                t = wp.tile([128, K], F32R, name=f"dpos_sb{k}", tag=f"dpos{k}")
                nc.sync.dma_start(t[:], dpos_d[rows, :])
                dpos_sb.append(t)
                t = wp.tile([128, K], F32R, name=f"dneg_sb{k}", tag=f"dneg{k}")
                nc.sync.dma_start(t[:], dneg_d[rows, :])
                dneg_sb.append(t)

            # --- phase 1: atdT = D @ actsT, then relu splits ---
            atd_sb = [[None] * NRC for _ in range(NK)]
            pos_sb = [[None] * NRC for _ in range(NK)]
            negeps_sb = [[None] * NRC for _ in range(NK)]
            for rc in range(NRC):
                for kp in range(NK):
                    cols = slice(kp * 128, (kp + 1) * 128)
                    ps = psp.tile([128, RC], F32, name=f"ps_atd{kp}_{rc}", tag="pn")
                    for d in range(ND):
                        nc.tensor.matmul(
                            ps[:],
                            DT_sb[d][:, cols],
                            acts_sb[d][rc][:],
                            start=(d == 0),
                            stop=(d == ND - 1),
                        )
                    atd = bigp.tile([128, RC], F32, name=f"atd{kp}_{rc}", tag=f"atd{kp}_{rc}")
                    nc.scalar.activation(atd[:], ps[:], Copy)
                    pos = bigp.tile([128, RC], F32R, name=f"pos{kp}_{rc}", tag=f"pos{kp}_{rc}")
                    nc.scalar.activation(pos[:], ps[:], Relu)
                    neg = tmpp.tile([128, RC], F32, name=f"neg{kp}_{rc}", tag="negt")
                    nc.scalar.activation(neg[:], ps[:], Relu, scale=-1.0)
                    nege = bigp.tile(
                        [128, RC], F32R, name=f"nege{kp}_{rc}", tag=f"nege{kp}_{rc}"
                    )
                    nc.vector.tensor_scalar_add(nege[:], neg[:], EPS)
                    atd_sb[kp][rc] = atd
                    pos_sb[kp][rc] = pos
                    negeps_sb[kp][rc] = nege

            # --- phase 2: z0T = max(inv @ atdT, eps) ---
            z_sb = [[[None] * NRC for _ in range(NK)] for _ in range(2)]
            for p in range(2):
                for k in range(NK):
                    for rc in range(NRC):
                        z_sb[p][k][rc] = zap.tile(
                            [128, RC], F32R, name=f"z{p}_{k}_{rc}", tag="za"
                        )
            for rc in range(NRC):
                for kp in range(NK):
                    cols = slice(kp * 128, (kp + 1) * 128)
                    ps = psp.tile([128, RC], F32, name=f"ps_z0{kp}_{rc}", tag="pd")
                    for k in range(NK):
                        nc.tensor.matmul(
                            ps[:],
                            inv_sb[k][:, cols],
                            atd_sb[k][rc][:],
                            start=(k == 0),
                            stop=(k == NK - 1),
                        )
                    nc.vector.tensor_scalar_max(z_sb[0][kp][rc][:], ps[:], EPS)

            # --- phase 3: multiplicative updates ---
            for t_it in range(n_iters):
                cur, nxt = t_it % 2, (t_it + 1) % 2
                for rc in range(NRC):
                    for kp in range(NK):
                        cols = slice(kp * 128, (kp + 1) * 128)
                        pn = psp.tile(
                            [128, RC], F32, name=f"pn{t_it}_{rc}_{kp}", tag="pn"
                        )
                        nc.tensor.matmul(
                            pn[:], eye_sb[:], pos_sb[kp][rc][:], start=True, stop=False
                        )
                        for k in range(NK):
                            nc.tensor.matmul(
                                pn[:],
                                dneg_sb[k][:, cols],
                                z_sb[cur][k][rc][:],
                                start=False,
                                stop=(k == NK - 1),
                            )
                        pd = psp.tile(
                            [128, RC], F32, name=f"pd{t_it}_{rc}_{kp}", tag="pd"
                        )
                        for k in range(NK):
                            nc.tensor.matmul(
                                pd[:],
                                dpos_sb[k][:, cols],
                                z_sb[cur][k][rc][:],
                                start=(k == 0),
                                stop=(k == NK - 1),
                            )
                        den = tmpp.tile(
                            [128, RC], F32, name=f"den{t_it}_{rc}_{kp}", tag="den"
                        )
                        nc.vector.tensor_add(den[:], pd[:], negeps_sb[kp][rc][:].bitcast(F32))
                        rcp = tmpp.tile(
                            [128, RC], F32, name=f"rcp{t_it}_{rc}_{kp}", tag="rcp"
                        )
                        nc.vector.reciprocal_approx_fast(rcp[:], den[:])
                        rat = tmpp.tile(
                            [128, RC], F32, name=f"rat{t_it}_{rc}_{kp}", tag="rat"
                        )
                        nc.vector.tensor_mul(rat[:], pn[:], rcp[:])
                        f = tmpp.tile([128, RC], F32, name=f"f{t_it}_{rc}_{kp}", tag="f")
                        nc.scalar.activation(f[:], rat[:], Sqrt)
                        nc.gpsimd.tensor_mul(
                            z_sb[nxt][kp][rc][:],
                            z_sb[cur][kp][rc][:].bitcast(F32),
                            f[:],
                        )

            # --- output ---
            fin = n_iters % 2
            for kp in range(NK):
                for rc in range(NRC):
                    nc.sync.dma_start(
                        out_d[kp * 128 : (kp + 1) * 128, rc * RC : (rc + 1) * RC],
                        z_sb[fin][kp][rc][:].bitcast(F32),
                    )

    nc.compile()
    return nc


def _get_program(n_iters: int):
    if n_iters not in _BUILD_CACHE:
        _BUILD_CACHE[n_iters] = _build(n_iters)
    return _BUILD_CACHE[n_iters]


def make_in_maps(acts: np.ndarray, D: np.ndarray):
    """Host-side sharding + kxk cache terms."""
    acts = np.ascontiguousarray(acts, dtype=np.float32)
    D = np.ascontiguousarray(D, dtype=np.float32)
    ddt = D @ D.T
    ddt_pos = ((np.abs(ddt) + ddt) * 0.5).astype(np.float32)
    ddt_neg = ((np.abs(ddt) - ddt) * 0.5).astype(np.float32)
    eye_k = np.eye(K, dtype=np.float32)
    inv = np.linalg.solve(ddt + np.float32(EPS) * eye_k, eye_k).astype(np.float32)
    DT = np.ascontiguousarray(D.T)
    actsT = np.ascontiguousarray(acts.T)
    eye128 = np.eye(128, dtype=np.float32)
    in_maps = []
    for c in range(N_CORES):
        in_maps.append(
            {
                "actsT": np.ascontiguousarray(actsT[:, c * R : (c + 1) * R]),
                "DT": DT,
                "ddt_pos": ddt_pos,
                "ddt_neg": ddt_neg,
                "ddt_inv": inv,
                "eye": eye128,
            }
        )
    return in_maps


def kernel(acts: np.ndarray, D: np.ndarray, n_iters) -> np.ndarray:
    n_iters = int(n_iters)
    nc = _get_program(n_iters)
    in_maps = make_in_maps(acts, D)
    # The update is NaN/Inf-free by construction (den >= eps, num >= 0), so a
    # non-finite output can only be transient execution corruption -> retry.
    for attempt in range(3):
        res = run_bass_kernel_spmd(nc, in_maps, core_ids=list(range(N_CORES)))
        z = np.empty((B, K), dtype=np.float32)
        for c in range(N_CORES):
            z[c * R : (c + 1) * R, :] = res.results[c]["zT"].T
        if np.isfinite(z).all():
            return z
    return z



# revision 10
# speedup vs baseline: 1.8373x; 1.0509x over previous
"""MiniBatchSemiNMF encode kernel for Trainium2 (8 NeuronCores, Bass/Tile).

Data-parallel over the batch (rows of `acts`): each of the 8 cores gets
1024 rows; D-derived k x k cache terms (ddt_pos, ddt_neg, (ddt+eps I)^-1)
are computed on the host (tiny: 512x512) and replicated to every core.

Device computation per core, in a transposed layout (k on partitions,
rows on the free dim), so no on-device transposes are needed:
    atdT  = D @ actsT                                  (PE, exact fp32)
    z0T   = max(inv @ atdT, eps)                       (PE + DVE)
    loop: numT = atd_posT + ddt_neg @ zT     (PE; identity-matmul folds the
          denT = atd_negT+eps + ddt_pos @ zT    atd_pos add into the PSUM
          zT  *= sqrt(numT / denT)              group; den add on DVE)
Iteration matmuls run as fp32r (full PE rate; ~12-bit mantissa products,
fp32 accumulate) -- iteration noise is contracted by the dynamics. The
atd/z0 matmuls run in exact fp32 (4x cycles, but only 96 of 1696 MMs):
their rounding would persist in z as an initial-condition error
(heavy cancellation in acts@D.T and atd@inv amplifies it ~20x).
"""

import sys

for _p in ("/opt/trn_rl_repo",):
    if _p not in sys.path:
        sys.path.insert(0, _p)

import numpy as np

import concourse.bacc as bacc
import concourse.tile as tile
from concourse import mybir
from concourse.bass_utils import run_bass_kernel_spmd

F32 = mybir.dt.float32
F32R = mybir.dt.float32r

EPS = 1e-8
N_CORES = 8
B, DM, K = 8192, 1024, 512  # batch, d_model, n_concepts
R = B // N_CORES  # rows per core (1024)
RC = 512  # row-chunk (moving-operand width)
NRC = R // RC  # 2 row chunks
NK = K // 128  # 4 k-tiles
ND = DM // 128  # 8 d-tiles

_BUILD_CACHE: dict[int, object] = {}  # v2: den-add on DVE


def _build(n_iters: int):
    """Build (and bacc-compile) the per-core Bass program."""
    nc = bacc.Bacc("TRN2", target_bir_lowering=False, debug=False, num_devices=N_CORES)

    actsT_d = nc.dram_tensor("actsT", [DM, R], F32, kind="ExternalInput").ap()
    DT_d = nc.dram_tensor("DT", [DM, K], F32, kind="ExternalInput").ap()
    dpos_d = nc.dram_tensor("ddt_pos", [K, K], F32R, kind="ExternalInput").ap()
    dneg_d = nc.dram_tensor("ddt_neg", [K, K], F32R, kind="ExternalInput").ap()
    inv_d = nc.dram_tensor("ddt_inv", [K, K], F32, kind="ExternalInput").ap()
    eye_d = nc.dram_tensor("eye", [128, 128], F32R, kind="ExternalInput").ap()
    out_d = nc.dram_tensor("zT", [K, R], F32, kind="ExternalOutput").ap()

    Relu = mybir.ActivationFunctionType.Relu
    Sqrt = mybir.ActivationFunctionType.Sqrt
    Rsqrt = mybir.ActivationFunctionType.Rsqrt
    Copy = mybir.ActivationFunctionType.Copy

    with tile.TileContext(nc) as tc:
        with (
            tc.tile_pool(name="weights", bufs=1) as wp,
            tc.tile_pool(name="big", bufs=1) as bigp,
            tc.tile_pool(name="zacts", bufs=2 * NK * NRC) as zap,
            tc.tile_pool(name="tmp", bufs=4) as tmpp,
            tc.tile_pool(name="psum", bufs=4, space="PSUM") as psp,
        ):
            # --- persistent weights ---
            eye_sb = wp.tile([128, 128], F32R, name="eye_sb", tag="eye")
            nc.sync.dma_start(eye_sb[:], eye_d[:])
            DT_sb = []
            acts_sb = [[None] * NRC for _ in range(ND)]
            for d in range(ND):
                t = wp.tile([128, K], F32, name=f"DT_sb{d}", tag=f"DT{d}")
                nc.sync.dma_start(t[:], DT_d[d * 128 : (d + 1) * 128, :])
                DT_sb.append(t)
                t = zap.tile([128, RC], F32, name=f"acts{d}_0", tag="za")
                nc.sync.dma_start(t[:], actsT_d[d * 128 : (d + 1) * 128, 0:RC])
                acts_sb[d][0] = t
            for d in range(ND):
                for rc in range(1, NRC):
                    t = zap.tile([128, RC], F32, name=f"acts{d}_{rc}", tag="za")
                    nc.sync.dma_start(
                        t[:], actsT_d[d * 128 : (d + 1) * 128, rc * RC : (rc + 1) * RC]
                    )
                    acts_sb[d][rc] = t
            inv_sb, dpos_sb, dneg_sb = [], [], []
            for k in range(NK):
                rows = slice(k * 128, (k + 1) * 128)
                t = wp.tile([128, K], F32, name=f"inv_sb{k}", tag=f"inv{k}")
                nc.sync.dma_start(t[:], inv_d[rows, :])
                inv_sb.append(t)
                t = wp.tile([128, K], F32R, name=f"dpos_sb{k}", tag=f"dpos{k}")
                nc.sync.dma_start(t[:], dpos_d[rows, :])
                dpos_sb.append(t)
                t = wp.tile([128, K], F32R, name=f"dneg_sb{k}", tag=f"dneg{k}")
                nc.sync.dma_start(t[:], dneg_d[rows, :])
                dneg_sb.append(t)

            # --- phase 1: atdT = D @ actsT, then relu splits ---
            atd_sb = [[None] * NRC for _ in range(NK)]
            pos_sb = [[None] * NRC for _ in range(NK)]
            negeps_sb = [[None] * NRC for _ in range(NK)]
            for rc in range(NRC):
                for kp in range(NK):
                    cols = slice(kp * 128, (kp + 1) * 128)
                    ps = psp.tile([128, RC], F32, name=f"ps_atd{kp}_{rc}", tag="pn")
                    for d in range(ND):
                        nc.tensor.matmul(
                            ps[:],
                            DT_sb[d][:, cols],
                            acts_sb[d][rc][:],
                            start=(d == 0),
                            stop=(d == ND - 1),
                        )
                    atd = bigp.tile([128, RC], F32, name=f"atd{kp}_{rc}", tag=f"atd{kp}_{rc}")
                    nc.scalar.activation(atd[:], ps[:], Copy)
                    pos = bigp.tile([128, RC], F32R, name=f"pos{kp}_{rc}", tag=f"pos{kp}_{rc}")
                    nc.scalar.activation(pos[:], ps[:], Relu)
                    neg = tmpp.tile([128, RC], F32, name=f"neg{kp}_{rc}", tag="negt")
                    nc.scalar.activation(neg[:], ps[:], Relu, scale=-1.0)
                    nege = bigp.tile(
                        [128, RC], F32R, name=f"nege{kp}_{rc}", tag=f"nege{kp}_{rc}"
                    )
                    nc.vector.tensor_scalar_add(nege[:], neg[:], EPS)
                    atd_sb[kp][rc] = atd
                    pos_sb[kp][rc] = pos
                    negeps_sb[kp][rc] = nege

            # --- phase 2: z0T = max(inv @ atdT, eps) ---
            z_sb = [[[None] * NRC for _ in range(NK)] for _ in range(2)]
            for p in range(2):
                for k in range(NK):
                    for rc in range(NRC):
                        z_sb[p][k][rc] = zap.tile(
                            [128, RC], F32R, name=f"z{p}_{k}_{rc}", tag="za"
                        )
            for rc in range(NRC):
                for kp in range(NK):
                    cols = slice(kp * 128, (kp + 1) * 128)
                    ps = psp.tile([128, RC], F32, name=f"ps_z0{kp}_{rc}", tag="pd")
                    for k in range(NK):
                        nc.tensor.matmul(
                            ps[:],
                            inv_sb[k][:, cols],
                            atd_sb[k][rc][:],
                            start=(k == 0),
                            stop=(k == NK - 1),
                        )
                    nc.vector.tensor_scalar_max(z_sb[0][kp][rc][:], ps[:], EPS)

            # --- phase 3: multiplicative updates ---
            for t_it in range(n_iters):
                cur, nxt = t_it % 2, (t_it + 1) % 2
                for rc in range(NRC):
                    for kp in range(NK):
                        cols = slice(kp * 128, (kp + 1) * 128)
                        pn = psp.tile(
                            [128, RC], F32, name=f"pn{t_it}_{rc}_{kp}", tag="pn"
                        )
                        nc.tensor.matmul(
                            pn[:], eye_sb[:], pos_sb[kp][rc][:], start=True, stop=False
                        )
                        for k in range(NK):
                            nc.tensor.matmul(
                                pn[:],
                                dneg_sb[k][:, cols],
                                z_sb[cur][k][rc][:],
                                start=False,
                                stop=(k == NK - 1),
                            )
                        pd = psp.tile(
                            [128, RC], F32, name=f"pd{t_it}_{rc}_{kp}", tag="pd"
                        )
                        for k in range(NK):
                            nc.tensor.matmul(
                                pd[:],
                                dpos_sb[k][:, cols],
                                z_sb[cur][k][rc][:],
                                start=(k == 0),
                                stop=(k == NK - 1),
                            )
                        den = tmpp.tile(
                            [128, RC], F32, name=f"den{t_it}_{rc}_{kp}", tag="den"
                        )
                        nc.vector.tensor_add(den[:], pd[:], negeps_sb[kp][rc][:].bitcast(F32))
                        rcp = tmpp.tile(
                            [128, RC], F32, name=f"rcp{t_it}_{rc}_{kp}", tag="rcp"
                        )
                        nc.vector.reciprocal_approx_fast(rcp[:], den[:])
                        rat = tmpp.tile(
                            [128, RC], F32, name=f"rat{t_it}_{rc}_{kp}", tag="rat"
                        )
                        nc.vector.tensor_mul(rat[:], pn[:], rcp[:])
                        f = tmpp.tile([128, RC], F32, name=f"f{t_it}_{rc}_{kp}", tag="f")
                        nc.scalar.activation(f[:], rat[:], Sqrt)
                        nc.gpsimd.tensor_mul(
                            z_sb[nxt][kp][rc][:],
                            z_sb[cur][kp][rc][:].bitcast(F32),
                            f[:],
                        )

            # --- output ---
            fin = n_iters % 2
            for kp in range(NK):
                for rc in range(NRC):
                    nc.sync.dma_start(
                        out_d[kp * 128 : (kp + 1) * 128, rc * RC : (rc + 1) * RC],
                        z_sb[fin][kp][rc][:].bitcast(F32),
                    )

    nc.compile()
    return nc


def _get_program(n_iters: int):
    if n_iters not in _BUILD_CACHE:
        _BUILD_CACHE[n_iters] = _build(n_iters)
    return _BUILD_CACHE[n_iters]


def make_in_maps(acts: np.ndarray, D: np.ndarray):
    """Host-side sharding + kxk cache terms."""
    acts = np.ascontiguousarray(acts, dtype=np.float32)
    D = np.ascontiguousarray(D, dtype=np.float32)
    ddt = D @ D.T
    ddt_pos = ((np.abs(ddt) + ddt) * 0.5).astype(np.float32)
    ddt_neg = ((np.abs(ddt) - ddt) * 0.5).astype(np.float32)
    eye_k = np.eye(K, dtype=np.float32)
    inv = np.linalg.solve(ddt + np.float32(EPS) * eye_k, eye_k).astype(np.float32)
    DT = np.ascontiguousarray(D.T)
    actsT = np.ascontiguousarray(acts.T)
    eye128 = np.eye(128, dtype=np.float32)
    in_maps = []
    for c in range(N_CORES):
        in_maps.append(
            {
                "actsT": np.ascontiguousarray(actsT[:, c * R : (c + 1) * R]),
                "DT": DT,
                "ddt_pos": ddt_pos,
                "ddt_neg": ddt_neg,
                "ddt_inv": inv,
                "eye": eye128,
            }
        )
    return in_maps


def kernel(acts: np.ndarray, D: np.ndarray, n_iters) -> np.ndarray:
    n_iters = int(n_iters)
    nc = _get_program(n_iters)
    in_maps = make_in_maps(acts, D)
    # The update is NaN/Inf-free by construction (den >= eps, num >= 0), so a
    # non-finite output can only be transient execution corruption; likewise a
    # raised device error (e.g. NRT_EXEC_UNIT_UNRECOVERABLE) is transient
    # infra state -> retry a couple of times before giving up.
    z = None
    last_exc = None
    for attempt in range(3):
        try:
            res = run_bass_kernel_spmd(nc, in_maps, core_ids=list(range(N_CORES)))
        except Exception as exc:  # noqa: BLE001 - device flake, retried
            last_exc = exc
            import time

            time.sleep(2.0 * (attempt + 1))
            continue
        z = np.empty((B, K), dtype=np.float32)
        for c in range(N_CORES):
            z[c * R : (c + 1) * R, :] = res.results[c]["zT"].T
        if np.isfinite(z).all():
            return z
    if z is None:
        raise last_exc
    return z



# revision 14
# speedup vs baseline: 1.8468x; 1.0052x over previous
"""MiniBatchSemiNMF encode kernel for Trainium2 (8 NeuronCores, Bass/Tile).

Data-parallel over the batch (rows of `acts`): each of the 8 cores gets
1024 rows; D-derived k x k cache terms (ddt_pos, ddt_neg, (ddt+eps I)^-1)
are computed on the host (tiny: 512x512) and replicated to every core.

Device computation per core, in a transposed layout (k on partitions,
rows on the free dim), so no on-device transposes are needed:
    atdT  = D @ actsT                                  (PE, fp16 hi/lo split)
    z0T   = max(inv @ atdT, eps)                       (PE + DVE)
    loop: numT = atd_posT + ddt_neg @ zT     (PE; identity-matmul folds the
          denT = atd_negT+eps + ddt_pos @ zT    atd_pos add into the PSUM
          zT  *= sqrt(numT / denT)              group; den add on DVE)
Iteration matmuls run as fp32r (full PE rate; ~12-bit mantissa products,
fp32 accumulate) -- iteration noise is contracted by the dynamics. The
atd/z0 matmuls need exact-fp32 accuracy (their rounding persists in z as
an initial-condition error; cancellation in acts@D.T and atd@inv
amplifies it ~20x) but run as fp16 hi/lo splits: 3 full-rate fp16 MMs
(hi@hi, and hi@lo'+lo'@hi at 2048x pre-scale in a second PSUM bank)
reproduce fp32 accuracy at 3 cycles/row vs fp32's 4 (fp16 11-bit
products are exact in the fp32 accumulator, unlike fp32r's).
"""

import sys

for _p in ("/opt/trn_rl_repo",):
    if _p not in sys.path:
        sys.path.insert(0, _p)

import numpy as np

import concourse.bacc as bacc
import concourse.tile as tile
from concourse import mybir
from concourse.bass_utils import run_bass_kernel_spmd

F32 = mybir.dt.float32
F32R = mybir.dt.float32r
F16 = mybir.dt.float16
LO_SCALE = 2048.0  # fp16 lo-term pre-scale (2**11) so cross products stay normal-range

EPS = 1e-8
N_CORES = 8
B, DM, K = 8192, 1024, 512  # batch, d_model, n_concepts
R = B // N_CORES  # rows per core (1024)
RC = 512  # row-chunk (moving-operand width)
NRC = R // RC  # 2 row chunks
NK = K // 128  # 4 k-tiles
ND = DM // 128  # 8 d-tiles

_BUILD_CACHE: dict[int, object] = {}  # v2: den-add on DVE


def _build(n_iters: int):
    """Build (and bacc-compile) the per-core Bass program."""
    nc = bacc.Bacc("TRN2", target_bir_lowering=False, debug=False, num_devices=N_CORES)

    actsTh_d = nc.dram_tensor("actsT_hi", [DM, R], F16, kind="ExternalInput").ap()
    actsTl_d = nc.dram_tensor("actsT_lo", [DM, R], F16, kind="ExternalInput").ap()
    DTh_d = nc.dram_tensor("DT_hi", [DM, K], F16, kind="ExternalInput").ap()
    DTl_d = nc.dram_tensor("DT_lo", [DM, K], F16, kind="ExternalInput").ap()
    dpos_d = nc.dram_tensor("ddt_pos", [K, K], F32R, kind="ExternalInput").ap()
    dneg_d = nc.dram_tensor("ddt_neg", [K, K], F32R, kind="ExternalInput").ap()
    invh_d = nc.dram_tensor("ddt_inv_hi", [K, K], F16, kind="ExternalInput").ap()
    invl_d = nc.dram_tensor("ddt_inv_lo", [K, K], F16, kind="ExternalInput").ap()
    eye_d = nc.dram_tensor("eye", [128, 128], F32R, kind="ExternalInput").ap()
    out_d = nc.dram_tensor("zT", [K, R], F32, kind="ExternalOutput").ap()

    Relu = mybir.ActivationFunctionType.Relu
    Sqrt = mybir.ActivationFunctionType.Sqrt
    Rsqrt = mybir.ActivationFunctionType.Rsqrt
    Copy = mybir.ActivationFunctionType.Copy

    with tile.TileContext(nc) as tc:
        with (
            tc.tile_pool(name="weights", bufs=1) as wp,
            tc.tile_pool(name="big", bufs=1) as bigp,
            tc.tile_pool(name="zpool", bufs=2 * NK * NRC) as zap,
            tc.tile_pool(name="actsp", bufs=2 * ND) as acp,
            tc.tile_pool(name="tmp", bufs=4) as tmpp,
            tc.tile_pool(name="psum", bufs=4, space="PSUM") as psp,
        ):
            # --- persistent weights ---
            eye_sb = wp.tile([128, 128], F32R, name="eye_sb", tag="eye")
            nc.sync.dma_start(eye_sb[:], eye_d[:])
            DTh_sb, DTl_sb = [], []
            acts_sb = [[None] * NRC for _ in range(ND)]  # (hi, lo) pairs
            for d in range(ND):
                th = wp.tile([128, K], F16, name=f"DTh_sb{d}", tag=f"DTh{d}")
                nc.sync.dma_start(th[:], DTh_d[d * 128 : (d + 1) * 128, :])
                DTh_sb.append(th)
                tl = wp.tile([128, K], F16, name=f"DTl_sb{d}", tag=f"DTl{d}")
                nc.sync.dma_start(tl[:], DTl_d[d * 128 : (d + 1) * 128, :])
                DTl_sb.append(tl)
                rows = slice(d * 128, (d + 1) * 128)
                ah = acp.tile([128, RC], F16, name=f"actsh{d}_0", tag="acts")
                nc.sync.dma_start(ah[:], actsTh_d[rows, 0:RC])
                al = acp.tile([128, RC], F16, name=f"actsl{d}_0", tag="acts")
                nc.sync.dma_start(al[:], actsTl_d[rows, 0:RC])
                acts_sb[d][0] = (ah, al)
            for rc in range(1, NRC):
                for d in range(ND):
                    rows = slice(d * 128, (d + 1) * 128)
                    cols = slice(rc * RC, (rc + 1) * RC)
                    ah = acp.tile([128, RC], F16, name=f"actsh{d}_{rc}", tag="acts")
                    nc.sync.dma_start(ah[:], actsTh_d[rows, cols])
                    al = acp.tile([128, RC], F16, name=f"actsl{d}_{rc}", tag="acts")
                    nc.sync.dma_start(al[:], actsTl_d[rows, cols])
                    acts_sb[d][rc] = (ah, al)
            invh_sb, invl_sb, dpos_sb, dneg_sb = [], [], [], []
            for k in range(NK):
                rows = slice(k * 128, (k + 1) * 128)
                t = wp.tile([128, K], F16, name=f"invh_sb{k}", tag=f"invh{k}")
                nc.sync.dma_start(t[:], invh_d[rows, :])
                invh_sb.append(t)
                t = wp.tile([128, K], F16, name=f"invl_sb{k}", tag=f"invl{k}")
                nc.sync.dma_start(t[:], invl_d[rows, :])
                invl_sb.append(t)
                t = wp.tile([128, K], F32R, name=f"dpos_sb{k}", tag=f"dpos{k}")
                nc.sync.dma_start(t[:], dpos_d[rows, :])
                dpos_sb.append(t)
                t = wp.tile([128, K], F32R, name=f"dneg_sb{k}", tag=f"dneg{k}")
                nc.sync.dma_start(t[:], dneg_d[rows, :])
                dneg_sb.append(t)

            # --- phase 1: atdT = D @ actsT via fp16 hi/lo split (3 cyc/row
            # of exact-fp32 accuracy instead of fp32's 4): psA = hi@hi,
            # psB = hi@lo' + lo'@hi with lo' pre-scaled by 2048, combined as
            # atd = psA + psB/2048. Also emits atd hi/lo fp16 splits for z0.
            atdh_sb = [[None] * NRC for _ in range(NK)]
            atdl_sb = [[None] * NRC for _ in range(NK)]
            pos_sb = [[None] * NRC for _ in range(NK)]
            negeps_sb = [[None] * NRC for _ in range(NK)]
            for rc in range(NRC):
                for kp in range(NK):
                    cols = slice(kp * 128, (kp + 1) * 128)
                    psA = psp.tile([128, RC], F32, name=f"psA_atd{kp}_{rc}", tag="pn")
                    for d in range(ND):
                        nc.tensor.matmul(
                            psA[:],
                            DTh_sb[d][:, cols],
                            acts_sb[d][rc][0][:],
                            start=(d == 0),
                            stop=(d == ND - 1),
                        )
                    psB = psp.tile([128, RC], F32, name=f"psB_atd{kp}_{rc}", tag="pd")
                    for d in range(ND):
                        nc.tensor.matmul(
                            psB[:],
                            DTh_sb[d][:, cols],
                            acts_sb[d][rc][1][:],
                            start=(d == 0),
                            stop=False,
                        )
                        nc.tensor.matmul(
                            psB[:],
                            DTl_sb[d][:, cols],
                            acts_sb[d][rc][0][:],
                            start=False,
                            stop=(d == ND - 1),
                        )
                    bs = tmpp.tile([128, RC], F32, name=f"bs{kp}_{rc}", tag="bs", bufs=2)
                    nc.scalar.activation(bs[:], psB[:], Copy, scale=1.0 / LO_SCALE)
                    atd = tmpp.tile([128, RC], F32, name=f"atd{kp}_{rc}", tag="atdt", bufs=2)
                    nc.vector.tensor_add(atd[:], psA[:], bs[:])
                    pos = bigp.tile([128, RC], F32R, name=f"pos{kp}_{rc}", tag=f"pos{kp}_{rc}")
                    nc.scalar.activation(pos[:], atd[:], Relu)
                    neg = tmpp.tile([128, RC], F32, name=f"neg{kp}_{rc}", tag="negt", bufs=2)
                    nc.scalar.activation(neg[:], atd[:], Relu, scale=-1.0)
                    nege = bigp.tile(
                        [128, RC], F32R, name=f"nege{kp}_{rc}", tag=f"nege{kp}_{rc}"
                    )
                    nc.vector.tensor_scalar_add(nege[:], neg[:], EPS)
                    atdh = bigp.tile([128, RC], F16, name=f"atdh{kp}_{rc}", tag=f"atdh{kp}_{rc}")
                    nc.scalar.activation(atdh[:], atd[:], Copy)
                    dres = tmpp.tile([128, RC], F32, name=f"dres{kp}_{rc}", tag="dres", bufs=2)
                    nc.vector.tensor_sub(dres[:], atd[:], atdh[:])
                    atdl = bigp.tile([128, RC], F16, name=f"atdl{kp}_{rc}", tag=f"atdl{kp}_{rc}")
                    nc.vector.tensor_scalar_mul(atdl[:], dres[:], LO_SCALE)
                    atdh_sb[kp][rc] = atdh
                    atdl_sb[kp][rc] = atdl
                    pos_sb[kp][rc] = pos
                    negeps_sb[kp][rc] = nege

            # --- phase 2: z0T = max(inv @ atdT, eps) ---
            z_sb = [[[None] * NRC for _ in range(NK)] for _ in range(2)]
            for p in range(2):
                for k in range(NK):
                    for rc in range(NRC):
                        z_sb[p][k][rc] = zap.tile(
                            [128, RC], F32R, name=f"z{p}_{k}_{rc}", tag="za"
                        )
            for rc in range(NRC):
                for kp in range(NK):
                    cols = slice(kp * 128, (kp + 1) * 128)
                    psA = psp.tile([128, RC], F32, name=f"psA_z0{kp}_{rc}", tag="pn")
                    for k in range(NK):
                        nc.tensor.matmul(
                            psA[:],
                            invh_sb[k][:, cols],
                            atdh_sb[k][rc][:],
                            start=(k == 0),
                            stop=(k == NK - 1),
                        )
                    psB = psp.tile([128, RC], F32, name=f"psB_z0{kp}_{rc}", tag="pd")
                    for k in range(NK):
                        nc.tensor.matmul(
                            psB[:],
                            invh_sb[k][:, cols],
                            atdl_sb[k][rc][:],
                            start=(k == 0),
                            stop=False,
                        )
                        nc.tensor.matmul(
                            psB[:],
                            invl_sb[k][:, cols],
                            atdh_sb[k][rc][:],
                            start=False,
                            stop=(k == NK - 1),
                        )
                    bs = tmpp.tile([128, RC], F32, name=f"bsz{kp}_{rc}", tag="bs", bufs=2)
                    nc.scalar.activation(bs[:], psB[:], Copy, scale=1.0 / LO_SCALE)
                    zt = tmpp.tile([128, RC], F32, name=f"zt{kp}_{rc}", tag="ztt", bufs=2)
                    nc.vector.tensor_add(zt[:], psA[:], bs[:])
                    nc.vector.tensor_scalar_max(z_sb[0][kp][rc][:], zt[:], EPS)

            # --- phase 3: multiplicative updates ---
            for t_it in range(n_iters):
                cur, nxt = t_it % 2, (t_it + 1) % 2
                for rc in range(NRC):
                    for kp in range(NK):
                        cols = slice(kp * 128, (kp + 1) * 128)
                        pn = psp.tile(
                            [128, RC], F32, name=f"pn{t_it}_{rc}_{kp}", tag="pn"
                        )
                        nc.tensor.matmul(
                            pn[:], eye_sb[:], pos_sb[kp][rc][:], start=True, stop=False
                        )
                        for k in range(NK):
                            nc.tensor.matmul(
                                pn[:],
                                dneg_sb[k][:, cols],
                                z_sb[cur][k][rc][:],
                                start=False,
                                stop=(k == NK - 1),
                            )
                        pd = psp.tile(
                            [128, RC], F32, name=f"pd{t_it}_{rc}_{kp}", tag="pd"
                        )
                        for k in range(NK):
                            nc.tensor.matmul(
                                pd[:],
                                dpos_sb[k][:, cols],
                                z_sb[cur][k][rc][:],
                                start=(k == 0),
                                stop=(k == NK - 1),
                            )
                        den = tmpp.tile(
                            [128, RC], F32, name=f"den{t_it}_{rc}_{kp}", tag="den"
                        )
                        nc.vector.tensor_add(den[:], pd[:], negeps_sb[kp][rc][:].bitcast(F32))
                        rcp = tmpp.tile(
                            [128, RC], F32, name=f"rcp{t_it}_{rc}_{kp}", tag="rcp"
                        )
                        nc.vector.reciprocal_approx_fast(rcp[:], den[:])
                        rat = tmpp.tile(
                            [128, RC], F32, name=f"rat{t_it}_{rc}_{kp}", tag="rat"
                        )
                        nc.vector.tensor_mul(rat[:], pn[:], rcp[:])
                        f = tmpp.tile([128, RC], F32, name=f"f{t_it}_{rc}_{kp}", tag="f")
                        nc.scalar.activation(f[:], rat[:], Sqrt)
                        nc.gpsimd.tensor_mul(
                            z_sb[nxt][kp][rc][:],
                            z_sb[cur][kp][rc][:].bitcast(F32),
                            f[:],
                        )

            # --- output ---
            fin = n_iters % 2
            for kp in range(NK):
                for rc in range(NRC):
                    nc.sync.dma_start(
                        out_d[kp * 128 : (kp + 1) * 128, rc * RC : (rc + 1) * RC],
                        z_sb[fin][kp][rc][:].bitcast(F32),
                    )

    nc.compile()
    return nc


def _get_program(n_iters: int):
    if n_iters not in _BUILD_CACHE:
        _BUILD_CACHE[n_iters] = _build(n_iters)
    return _BUILD_CACHE[n_iters]


def make_in_maps(acts: np.ndarray, D: np.ndarray):
    """Host-side sharding + kxk cache terms."""
    acts = np.ascontiguousarray(acts, dtype=np.float32)
    D = np.ascontiguousarray(D, dtype=np.float32)
    ddt = D @ D.T
    ddt_pos = ((np.abs(ddt) + ddt) * 0.5).astype(np.float32)
    ddt_neg = ((np.abs(ddt) - ddt) * 0.5).astype(np.float32)
    eye_k = np.eye(K, dtype=np.float32)
    inv = np.linalg.solve(ddt + np.float32(EPS) * eye_k, eye_k).astype(np.float32)

    def split16(x):
        hi = x.astype(np.float16)
        lo = ((x - hi.astype(np.float32)) * np.float32(LO_SCALE)).astype(np.float16)
        return hi, lo

    DT_hi, DT_lo = split16(np.ascontiguousarray(D.T))
    inv_hi, inv_lo = split16(inv)
    actsT = np.ascontiguousarray(acts.T)
    eye128 = np.eye(128, dtype=np.float32)
    in_maps = []
    for c in range(N_CORES):
        a_hi, a_lo = split16(np.ascontiguousarray(actsT[:, c * R : (c + 1) * R]))
        in_maps.append(
            {
                "actsT_hi": a_hi,
                "actsT_lo": a_lo,
                "DT_hi": DT_hi,
                "DT_lo": DT_lo,
                "ddt_pos": ddt_pos,
                "ddt_neg": ddt_neg,
                "ddt_inv_hi": inv_hi,
                "ddt_inv_lo": inv_lo,
                "eye": eye128,
            }
        )
    return in_maps


def kernel(acts: np.ndarray, D: np.ndarray, n_iters) -> np.ndarray:
    n_iters = int(n_iters)
    nc = _get_program(n_iters)
    in_maps = make_in_maps(acts, D)
    # The update is NaN/Inf-free by construction (den >= eps, num >= 0), so a
    # non-finite output can only be transient execution corruption; likewise a
    # raised device error (e.g. NRT_EXEC_UNIT_UNRECOVERABLE) is transient
    # infra state -> retry a couple of times before giving up.
    z = None
    last_exc = None
    for attempt in range(3):
        try:
            res = run_bass_kernel_spmd(nc, in_maps, core_ids=list(range(N_CORES)))
        except Exception as exc:  # noqa: BLE001 - device flake, retried
            last_exc = exc
            import time

            time.sleep(2.0 * (attempt + 1))
            continue
        z = np.empty((B, K), dtype=np.float32)
        for c in range(N_CORES):
            z[c * R : (c + 1) * R, :] = res.results[c]["zT"].T
        if np.isfinite(z).all():
            return z
    if z is None:
        raise last_exc
    return z

